# revision 15
# baseline (speedup 1.0000x reference)
"""Trainium2 Bass kernel for nn_DIVLoss (retrieval_knn).

Math: the reference's pred_nn = mean(pred_nn_mat @ nn_label_matrix, axis=1)
collapses exactly (each row of nn_label_matrix holds exactly 10 ones), so
    pred_nn[i] = (10/B) * fsum . qhat[target[i]],   fsum = sum_b fhat[b]
    pred_sel[i] = fhat[perm[i]] . qhat[target[perm[i]]],  perm = stable argsort
    loss = mean_i softplus(SCALE * (pred_nn[i] - pred_sel[i]))

Device/host split: the device does the O(B*D) dot products; the host does
routing (argsort/gathers/transposes), norms, fsum, and the final
softplus+mean over 4096 scalars.

Key structure: rows are shipped perm-sorted, so a core's 512 rows span only
~125 consecutive classes (window CW=160) and each 128-row tile fits a
96-class window on a fixed cross-core grid. Both the sel and nn paths then
share ONE small qhat-window matrix W [1024, CW] per core:
  - TensorE computes P_t = xs_t @ W[:, g_t:g_t+96] (4 tiles, fp8,
    8 accumulated chunk matmuls each, stationary = transposed x chunks)
    and v = (UN*fsum) @ W as a separate accumulation.
  - VectorE extracts z_sel per row as an STT with a one-hot fp8 mask:
    du[:,t] = sum_c P_t[i,c]*M_t[i,c] = P_t[i, rel(i)]  (exact).
  - ScalarE only copies v PSUM->SBUF (no activations -> no act table).
  - Dummy matmuls at body start keep the PE busy through the DMA wait so
    the HAM clock gate releases (1.2 -> 2.4 GHz) before the real matmuls.
Per-core input is ~768KB (vs 1.58MB for the per-row form): wv 208KB +
4 x 128KB transposed x tiles + 48KB masks. Scales are powers of two and
divide out exactly on the host (z = P/32).
"""

import numpy as np

N_CORES = 8
B = 4096
D = 1024
C = 1000
ROWS = B // N_CORES          # 512 rows per core
T = ROWS // 128              # 4 row-tiles of 128 partitions
CH = D // 128                # 8 contraction chunks
CW = 160                     # per-core class window (max span ~134)
TW = 96                      # per-tile class window on the shared grid
SCALE = 100.0
TOPK = 10.0
UN = SCALE * TOPK / B        # nn-path constant folded into fsum
PAD = 64                     # fsum in wv[:, 0:8); W chunks 64B-aligned

_cache = {}


def _build(grid):
    """grid: T compile-time tile-window offsets shared by all cores."""
    import concourse.bacc as bacc
    import concourse.mybir as mybir
    import concourse.tile as tile

    f32 = mybir.dt.float32
    f8 = mybir.dt.float8e4
    bf16 = mybir.dt.bfloat16
    ALU = mybir.AluOpType

    nc = bacc.Bacc(
        "TRN2",
        target_bir_lowering=False,
        debug=False,
        enable_asserts=False,
        num_devices=N_CORES,
    )

    WCOLS = PAD + CH * CW
    wv_d = nc.dram_tensor("wv", [128, WCOLS], f8, kind="ExternalInput")
    xt_d = [
        nc.dram_tensor(f"xt{t}", [128, CH * 128], f8, kind="ExternalInput")
        for t in range(T)
    ]
    mk_d = nc.dram_tensor("mk", [128, T * TW], f8, kind="ExternalInput")
    du_d = nc.dram_tensor("du", [128, T], f32, kind="ExternalOutput")
    uo_d = nc.dram_tensor("uo", [1, CW], f32, kind="ExternalOutput")

    with tile.TileContext(nc) as tc:
        with tc.tile_pool(name="sbuf", bufs=1) as pool, tc.tile_pool(
            name="ps", space="PSUM", bufs=1
        ) as pp:
            wv = pool.tile([128, WCOLS], f8, tag="wv")
            xt = [
                pool.tile([128, CH * 128], f8, name=f"xt{t}", tag=f"xt{t}")
                for t in range(T)
            ]
            mk = pool.tile([128, T * TW], f8, tag="mk")
            dmy = pool.tile([128, 512], bf16, tag="dmy")
            du = pool.tile([128, T], f32, tag="du")
            usb = pool.tile([1, CW], f32, tag="usb")
            prodx = pool.tile([128, TW], f32, tag="prodx")
            pu = pp.tile([1, CW], f32, name="pu", tag="pu")
            pdm = pp.tile([1, 512], f32, name="pdm", tag="pdm")
            pt = [
                pp.tile([128, 512], f32, name=f"pt{t}", tag=f"pt{t}")
                for t in range(T)
            ]

            # inputs: interleaved across BOTH HWDGE rings in consumption
            # order — one ring alone is capped at ~185GB/s by the early
            # activity throttle, two together reach ~250GB/s.
            nc.sync.dma_start(wv[:], wv_d[:])
            nc.scalar.dma_start(mk[:], mk_d[:])
            nc.sync.dma_start(xt[0][:], xt_d[0][:])
            nc.scalar.dma_start(xt[1][:], xt_d[1][:])
            nc.sync.dma_start(xt[2][:], xt_d[2][:])
            nc.scalar.dma_start(xt[3][:], xt_d[3][:])

            # PE warm-up: keep the array busy from body start so the HAM
            # clock gate releases before the real matmuls arrive.
            nc.vector.memset(dmy[:], 0.0)
            for i in range(4):
                nc.tensor.matmul(pdm[:], dmy[:, i : i + 1], dmy[:],
                                 start=True, stop=True)

            # nn path: v = fsw . W  (PSUM [1, CW]); 1-wide stationary, so
            # plain matmuls (DoubleRow's [128,2,1] lhsT fails the ISA check)
            NP = CH // 2
            for c in range(CH):
                nc.tensor.matmul(
                    pu[:],
                    wv[:, c : c + 1],
                    wv[:, PAD + c * CW : PAD + (c + 1) * CW],
                    start=(c == 0),
                    stop=(c == CH - 1),
                )

            # sel path: P_t = xs_t @ W[:, g_t : g_t+TW], DoubleRow pairs
            for t in range(T):
                for p in range(NP):
                    lhsT = xt[t][
                        :, 2 * p * 128 : (2 * p + 2) * 128
                    ].rearrange("p (two f) -> p two f", two=2)
                    rhs = wv[:, PAD + 2 * p * CW : PAD + (2 * p + 2) * CW].rearrange(
                        "p (two f) -> p two f", two=2
                    )[:, :, grid[t] : grid[t] + TW]
                    nc.tensor.matmul(
                        pt[t][:, 0:TW],
                        lhsT,
                        rhs,
                        start=(p == 0),
                        stop=(p == NP - 1),
                        perf_mode=mybir.MatmulPerfMode.DoubleRow,
                    )

            # VectorE: exact one-hot extraction -> du[:, t]
            for t in range(T):
                nc.vector.scalar_tensor_tensor(
                    prodx[:], pt[t][:, 0:TW], 1.0, mk[:, t * TW : (t + 1) * TW],
                    ALU.mult, ALU.mult, accum_out=du[:, t : t + 1],
                )

            # ScalarE: v PSUM -> SBUF, then out
            nc.scalar.copy(usb[:], pu[:])
            nc.scalar.dma_start(uo_d[:], usb[:])
            nc.sync.dma_start(du_d[:], du[:])

    nc.compile()
    return nc


def _host_prep(feature, query, target):
    import ml_dtypes

    f8 = ml_dtypes.float8_e4m3

    f = feature.astype(np.float64)
    q = query.astype(np.float64)
    t = np.asarray(target).astype(np.int64)
    perm = np.argsort(t, kind="stable")
    ts = t[perm]

    nf = np.sqrt((f * f).sum(1))
    nq = np.sqrt((q * q).sum(1))
    qhat = q / nq[:, None]
    fsum = (f / nf[:, None]).sum(0)

    xs = (f[perm] / nf[perm, None]) * SCALE      # SCALE * fhat, perm order
    x8 = np.ascontiguousarray(xs.astype(f8))

    qh8 = (qhat * 32.0).astype(f8)               # 2^5 folded, shared W
    fsb8 = (fsum * UN).astype(f8)
    fsw = np.zeros((128, PAD), dtype=f8)
    fsw[:, 0:CH] = fsb8.reshape(CH, 128).T

    # per-core window bases + the shared per-tile grid
    bases = []
    lo_kt = np.zeros((N_CORES, T), dtype=np.int64)
    hi_kt = np.zeros((N_CORES, T), dtype=np.int64)
    for k in range(N_CORES):
        seg = ts[k * ROWS : (k + 1) * ROWS]
        lo, hi = int(seg[0]), int(seg[-1])
        assert hi - lo + 1 <= CW, (lo, hi)
        base = min(lo, C - CW)
        bases.append(base)
        for tt in range(T):
            tseg = seg[tt * 128 : (tt + 1) * 128]
            lo_kt[k, tt] = int(tseg[0]) - base
            hi_kt[k, tt] = int(tseg[-1]) - base
    grid = []
    for tt in range(T):
        gmin = max(0, int(hi_kt[:, tt].max()) - TW + 1)
        gmax = min(CW - TW, int(lo_kt[:, tt].min()))
        assert gmin <= gmax, (tt, gmin, gmax)
        grid.append((gmin + gmax) // 2)

    rel = np.empty(B, dtype=np.int64)
    for k in range(N_CORES):
        for tt in range(T):
            rows = slice(k * ROWS + tt * 128, k * ROWS + (tt + 1) * 128)
            rel[rows] = ts[rows] - bases[k] - grid[tt]
    assert rel.min() >= 0 and rel.max() < TW
    return x8, qh8, fsw, bases, tuple(grid), rel, t


def kernel(feature, query, target):
    feature = np.ascontiguousarray(np.asarray(feature), dtype=np.float32)
    query = np.ascontiguousarray(np.asarray(query), dtype=np.float32)
    target = np.asarray(target)

    x8, qh8, fsw, bases, grid, rel, t = _host_prep(feature, query, target)

    if grid not in _cache:
        _cache[grid] = _build(grid)
    nc = _cache[grid]

    import ml_dtypes
    f8d = np.dtype(ml_dtypes.float8_e4m3)

    ridx = np.arange(128)
    in_maps = []
    for k in range(N_CORES):
        s0 = k * ROWS
        wk = qh8[bases[k] : bases[k] + CW]            # [CW, 1024]
        chunks = np.ascontiguousarray(wk.T).reshape(CH, 128, CW)
        wvrow = np.concatenate(
            [fsw.view(np.uint8)]
            + [np.ascontiguousarray(chunks[c]).view(np.uint8) for c in range(CH)],
            axis=1,
        )

        imap = {"wv": np.ascontiguousarray(wvrow).view(f8d)}
        mks = []
        for tt in range(T):
            rows = slice(s0 + tt * 128, s0 + (tt + 1) * 128)
            xtT = np.ascontiguousarray(x8[rows].view(np.uint8).T)  # [1024,128]
            imap[f"xt{tt}"] = np.ascontiguousarray(
                xtT.reshape(CH, 128, 128).transpose(1, 0, 2).reshape(128, CH * 128)
            ).view(f8d)
            m = np.zeros((128, TW), dtype=f8d)
            m[ridx, rel[rows]] = 1.0
            mks.append(m.view(np.uint8))
        imap["mk"] = np.ascontiguousarray(np.concatenate(mks, axis=1)).view(f8d)
        in_maps.append(imap)

    from concourse.bass_utils import run_bass_kernel_spmd

    res = run_bass_kernel_spmd(
        nc,
        in_maps,
        core_ids=list(range(N_CORES)),
        trace=bool(getattr(kernel, "_trace", False)),
        tmpdir=getattr(kernel, "_tmpdir", None),
    )
    kernel.last_results = res

    z_sel = np.empty(B)
    v_full = np.zeros(C)
    for k in range(N_CORES):
        s0 = k * ROWS
        du = res.results[k]["du"].astype(np.float64)   # [128, T]
        uo = res.results[k]["uo"].astype(np.float64)   # [1, CW]
        v_full[bases[k] : bases[k] + CW] = uo[0] / 32.0
        for tt in range(T):
            rows = slice(s0 + tt * 128, s0 + (tt + 1) * 128)
            z_sel[rows] = du[:, tt] / 32.0

    # z_sel is in perm-row order; z_nn in original order — matching the
    # reference's own (faithfully replicated) row pairing.
    z_nn = v_full[t]
    loss = np.mean(np.logaddexp(0.0, z_nn - z_sel))
    return np.asarray(loss, dtype=np.float32)
